# revision 10
# baseline (speedup 1.0000x reference)
"""GAU (Gated Attention Unit) encoder kernel for Trainium2, 8 NeuronCores.

Reference computation (per sample, B=8 samples total, one per core):
    xn   = ScaleNorm(x) * g                          # [K, D]
    uv   = silu(xn @ uv_w.T)                         # [K, 2E+S]
    u, v, base = split(uv, [E, E, S])
    q, k = base * gamma[i] + beta[i]                 # [K, S] each
    kern = relu(q @ k.T / sqrt(S))^2                 # [K, K]
    out  = (u * (kern @ v)) @ o_w.T + x * res_scale  # [K, D]

Sharding: data-parallel over batch B — one sample per NeuronCore (SPMD,
no collectives). Weights replicated.

Layout strategy (per core):
  - x arrives token-major [K, D]; ScaleNorm is a free-dim reduction.
  - xn is transposed on-chip via PE (64 128x128 transposes) to xnT [D, K],
    which feeds every uv-projection matmul (contraction over D needs D on
    partitions).
  - q, k, base, u are produced feature-major ([S|E, K]); v token-major
    [K, E]. Scores are computed transposed (scoresT [k, q]) so the
    aggregation matmul can contract over k with no further transposes, and
    the gated output (feature-major) directly feeds the output projection
    as the stationary operand, yielding token-major out tiles.
  - Matmuls use fp32r (full PE rate, ~1e-3 relative accuracy) except the
    big aggregation matmul which uses bf16 operands (v, kern) for SBUF
    headroom.
"""

import numpy as np

import concourse.bass as bass
import concourse.tile as tile
from concourse import bacc, mybir
from concourse.bass_utils import run_bass_kernel_spmd
from concourse.masks import make_identity

F32 = mybir.dt.float32
F32R = mybir.dt.float32r
BF16 = mybir.dt.bfloat16
AF = mybir.ActivationFunctionType
OP = mybir.AluOpType

B, K, D = 8, 2048, 512
E, S = 1024, 128
F = 2 * E + S  # 2176
EPS = 1e-5
P = 128
KT = K // P    # 16 token tiles
DT = D // P    # 4  d tiles
ET = E // P    # 8  e tiles
QB = K // 512  # 4  q blocks of 512 tokens
N_CORES = 8

# dtype of the aggregation matmul operands (kern, v)
AGG_DT = BF16


def gau_tile_kernel(ctx, tc, out_d, x_d, uvwT_d, owT_d, gbT_d, rs_d, g_val, dbg=None):
    nc = tc.nc
    inv_sqrt_s = 1.0 / float(np.sqrt(S))

    const = ctx.enter_context(tc.tile_pool(name="const", bufs=1))
    persist = ctx.enter_context(tc.tile_pool(name="persist", bufs=1))
    xwork = ctx.enter_context(tc.tile_pool(name="xwork", bufs=2))
    tmps = ctx.enter_context(tc.tile_pool(name="tmps", bufs=4))
    attn = ctx.enter_context(tc.tile_pool(name="attn", bufs=1))
    owork = ctx.enter_context(tc.tile_pool(name="owork", bufs=2))
    ps_t = ctx.enter_context(tc.tile_pool(name="ps_t", bufs=2, space="PSUM"))
    ps_mm = ctx.enter_context(tc.tile_pool(name="ps_mm", bufs=4, space="PSUM"))

    # ---- constants / weights ----
    ident = const.tile([P, P], F32)
    make_identity(nc, ident)
    gbT = const.tile([P, 4], F32)  # cols: gamma0, gamma1, beta0, beta1
    nc.sync.dma_start(gbT[:], gbT_d)
    rs_b = const.tile([P, D], F32)  # res_scale broadcast across partitions
    nc.sync.dma_start(rs_b[:], rs_d.partition_broadcast(P))

    uvw_r = uvwT_d.rearrange("(po pi) f -> pi po f", pi=P)  # [128, 4, 2176]
    uvw_u = persist.tile([P, DT, E], F32R)
    nc.sync.dma_start(uvw_u[:], uvw_r[:, :, 0:E])
    ow_r = owT_d.rearrange("(po pi) d -> pi po d", pi=P)  # [128, 8, 512]
    o_wT = persist.tile([P, ET, D], F32R)
    nc.sync.dma_start(o_wT[:], ow_r)

    xnT = persist.tile([P, DT, K], F32R)
    qT = persist.tile([P, K], F32R)
    kTt = persist.tile([P, K], F32R)
    v_sb = persist.tile([P, KT, E], AGG_DT)

    with tc.tile_pool(name="uv_vb", bufs=1) as uvvb_pool:
        uvw_vb = uvvb_pool.tile([P, DT, E + S], F32R)
        nc.sync.dma_start(uvw_vb[:], uvw_r[:, :, E:F])

        # ---- phase 1: ScaleNorm + transpose -> xnT ----
        for i in range(KT):
            x_i = xwork.tile([P, D], F32, tag="x_in")
            nc.sync.dma_start(x_i[:], x_d[i * P : (i + 1) * P, :])
            st = tmps.tile([P, nc.vector.BN_STATS_DIM], F32, tag="bn")
            nc.vector.bn_stats(out=st[:], in_=x_i[:])
            mv = tmps.tile([P, nc.vector.BN_AGGR_DIM], F32, tag="mv")
            nc.vector.bn_aggr(out=mv[:], in_=st[:])
            # meansq = mean^2 + var ; norm = sqrt(meansq)
            msq = tmps.tile([P, 1], F32, tag="msq")
            nc.vector.tensor_tensor(msq[:], mv[:, 0:1], mv[:, 0:1], OP.mult)
            nc.vector.tensor_tensor(msq[:], msq[:], mv[:, 1:2], OP.add)
            nrm = tmps.tile([P, 1], F32, tag="nrm")
            nc.scalar.activation(nrm[:], msq[:], AF.Sqrt)
            nc.vector.tensor_scalar_max(nrm[:], nrm[:], EPS)
            rn = tmps.tile([P, 1], F32, tag="rn")
            nc.vector.reciprocal(rn[:], nrm[:])
            xn_i = xwork.tile([P, D], F32, tag="xn")
            nc.vector.tensor_scalar(
                xn_i[:], x_i[:], rn[:], float(g_val), op0=OP.mult, op1=OP.mult
            )
            for j in range(DT):
                pt = ps_t.tile([P, P], F32)
                nc.tensor.transpose(pt[:], xn_i[:, j * P : (j + 1) * P], ident[:])
                nc.scalar.activation(
                    xnT[:, j, i * P : (i + 1) * P], pt[:], AF.Copy
                )

        # ---- phase 2a: base -> qT, kT (feature-major [S, K]) ----
        for nb in range(QB):
            pb = ps_mm.tile([P, 512], F32, tag="mm")
            for j in range(DT):
                nc.tensor.matmul(
                    pb[:],
                    uvw_vb[:, j, E : E + S],
                    xnT[:, j, nb * 512 : (nb + 1) * 512],
                    start=(j == 0),
                    stop=(j == DT - 1),
                )
            sl = slice(nb * 512, (nb + 1) * 512)
            bs = owork.tile([P, 512], F32, tag="bs")
            nc.scalar.activation(bs[:], pb[:], AF.Silu)
            nc.vector.tensor_scalar(
                qT[:, sl], bs[:], gbT[:, 0:1], gbT[:, 2:3], op0=OP.mult, op1=OP.add
            )
            nc.vector.tensor_scalar(
                kTt[:, sl], bs[:], gbT[:, 1:2], gbT[:, 3:4], op0=OP.mult, op1=OP.add
            )

        # ---- phase 2b: v token-major [K, E], silu, cast ----
        for i in range(KT):
            for nb2 in range(2):
                pv = ps_mm.tile([P, 512], F32, tag="mm")
                for j in range(DT):
                    nc.tensor.matmul(
                        pv[:],
                        xnT[:, j, i * P : (i + 1) * P],
                        uvw_vb[:, j, nb2 * 512 : (nb2 + 1) * 512],
                        start=(j == 0),
                        stop=(j == DT - 1),
                    )
                nc.scalar.activation(
                    v_sb[:, i, nb2 * 512 : (nb2 + 1) * 512], pv[:], AF.Silu
                )

    if dbg is not None:
        nc.gpsimd.dma_start(dbg["xnT"], xnT[:])
        nc.gpsimd.dma_start(dbg["qT"], qT[:])
        nc.gpsimd.dma_start(dbg["kT"], kTt[:])
        nc.gpsimd.dma_start(dbg["v"], v_sb[:])

    # ---- phase 3: attention, per q-block of 512 tokens ----
    for qb in range(QB):
        qsl = slice(qb * 512, (qb + 1) * 512)
        # u for this q-block (feature-major [E, 512]), silu
        u_qb = attn.tile([P, ET, 512], F32, tag="u")
        for uf in range(ET):
            pu = ps_mm.tile([P, 512], F32, tag="mm")
            for j in range(DT):
                nc.tensor.matmul(
                    pu[:],
                    uvw_u[:, j, uf * P : (uf + 1) * P],
                    xnT[:, j, qsl],
                    start=(j == 0),
                    stop=(j == DT - 1),
                )
            nc.scalar.activation(u_qb[:, uf, :], pu[:], AF.Silu)

        # scoresT [k, q] -> relu(.)/sqrt(S) -> square -> kern (bf16)
        kern = attn.tile([P, KT, 512], AGG_DT, tag="kern")
        for kt in range(KT):
            psc = ps_mm.tile([P, 512], F32, tag="mm")
            nc.tensor.matmul(
                psc[:],
                kTt[:, kt * P : (kt + 1) * P],
                qT[:, qsl],
                start=True,
                stop=True,
            )
            rt = tmps.tile([P, 512], AGG_DT, tag="relu")
            nc.scalar.activation(rt[:], psc[:], AF.Relu, scale=inv_sqrt_s)
            nc.vector.tensor_tensor(kern[:, kt, :], rt[:], rt[:], OP.mult)

        if dbg is not None and qb == 0:
            nc.gpsimd.dma_start(dbg["u0"], u_qb[:])
            nc.gpsimd.dma_start(dbg["kern0"], kern[:])

        # aggT [e, q] += v.T-slices @ kern ; gate with u
        gated = attn.tile([P, ET, 512], F32R, tag="gated")
        for et in range(ET):
            pa = ps_mm.tile([P, 512], F32, tag="mm")
            for kt in range(KT):
                nc.tensor.matmul(
                    pa[:],
                    v_sb[:, kt, et * P : (et + 1) * P],
                    kern[:, kt, :],
                    start=(kt == 0),
                    stop=(kt == KT - 1),
                )
            nc.vector.tensor_tensor(gated[:, et, :], u_qb[:, et, :], pa[:], OP.mult)

        if dbg is not None and qb == 0:
            nc.gpsimd.dma_start(dbg["gated0"], gated[:])

        # output projection + residual, token-major
        for tq in range(4):
            i = qb * 4 + tq
            po = ps_mm.tile([P, 512], F32, tag="mm")
            for et in range(ET):
                nc.tensor.matmul(
                    po[:],
                    gated[:, et, tq * P : (tq + 1) * P],
                    o_wT[:, et, :],
                    start=(et == 0),
                    stop=(et == ET - 1),
                )
            x_r = owork.tile([P, D], F32, tag="x_res")
            nc.sync.dma_start(x_r[:], x_d[i * P : (i + 1) * P, :])
            ot = owork.tile([P, D], F32, tag="out")
            nc.vector.tensor_tensor(ot[:], x_r[:], rs_b[:], OP.mult)
            nc.vector.tensor_tensor(ot[:], ot[:], po[:], OP.add)
            nc.sync.dma_start(out_d[i * P : (i + 1) * P, :], ot[:])


def build_program(g_val):
    nc = bacc.Bacc("TRN2", target_bir_lowering=False, debug=False, num_devices=N_CORES)
    x_d = nc.dram_tensor("x", [K, D], F32, kind="ExternalInput").ap()
    uvwT_d = nc.dram_tensor("uvw_t", [D, F], F32R, kind="ExternalInput").ap()
    owT_d = nc.dram_tensor("ow_t", [E, D], F32R, kind="ExternalInput").ap()
    gbT_d = nc.dram_tensor("gb_t", [P, 4], F32, kind="ExternalInput").ap()
    rs_d = nc.dram_tensor("res_scale", [D], F32, kind="ExternalInput").ap()
    out_d = nc.dram_tensor("out", [K, D], F32, kind="ExternalOutput").ap()

    from contextlib import ExitStack

    with tile.TileContext(nc) as tc, ExitStack() as ctx:
        gau_tile_kernel(ctx, tc, out_d, x_d, uvwT_d, owT_d, gbT_d, rs_d, g_val)
    nc.compile()
    return nc


_PROGRAM_CACHE = {}


def _get_program(g_val):
    key = float(g_val)
    if key not in _PROGRAM_CACHE:
        _PROGRAM_CACHE[key] = build_program(key)
    return _PROGRAM_CACHE[key]


def make_in_maps(x, uv_w, o_w, gamma, beta, res_scale):
    uvwT = np.ascontiguousarray(uv_w.T.astype(np.float32))  # [D, F]
    owT = np.ascontiguousarray(o_w.T.astype(np.float32))  # [E, D]
    gbT = np.ascontiguousarray(
        np.stack([gamma[0], gamma[1], beta[0], beta[1]], axis=1).astype(np.float32)
    )  # [S, 4]
    rs = np.ascontiguousarray(res_scale.astype(np.float32))
    return [
        {
            "x": np.ascontiguousarray(x[b].astype(np.float32)),
            "uvw_t": uvwT,
            "ow_t": owT,
            "gb_t": gbT,
            "res_scale": rs,
        }
        for b in range(N_CORES)
    ]


def kernel(x, uv_w, o_w, gamma, beta, g, res_scale):
    x = np.asarray(x)
    nc = _get_program(float(np.asarray(g).reshape(-1)[0]))
    in_maps = make_in_maps(
        x,
        np.asarray(uv_w),
        np.asarray(o_w),
        np.asarray(gamma),
        np.asarray(beta),
        np.asarray(res_scale),
    )
    res = run_bass_kernel_spmd(nc, in_maps, core_ids=list(range(N_CORES)))
    out = np.stack([r["out"] for r in res.results], axis=0)
    return out.astype(x.dtype)


# revision 17
# speedup vs baseline: 17.8476x; 17.8476x over previous
"""GAU (Gated Attention Unit) encoder kernel for Trainium2, 8 NeuronCores.

Reference computation (per sample, B=8 samples total, one per core):
    xn   = ScaleNorm(x) * g                          # [K, D]
    uv   = silu(xn @ uv_w.T)                         # [K, 2E+S]
    u, v, base = split(uv, [E, E, S])
    q, k = base * gamma[i] + beta[i]                 # [K, S] each
    kern = relu(q @ k.T / sqrt(S))^2                 # [K, K]
    out  = (u * (kern @ v)) @ o_w.T + x * res_scale  # [K, D]

Sharding: data-parallel over batch B — one sample per NeuronCore (SPMD,
no collectives). Weights replicated.

Layout strategy (per core):
  - x arrives token-major [K, D]; ScaleNorm is a free-dim reduction.
  - xn is transposed on-chip via PE (64 128x128 transposes) to xnT [D, K],
    which feeds every uv-projection matmul (contraction over D needs D on
    partitions).
  - q, k, base, u are produced feature-major ([S|E, K]); v token-major
    [K, E]. Scores are computed transposed (scoresT [k, q]) so the
    aggregation matmul can contract over k with no further transposes, and
    the gated output (feature-major) directly feeds the output projection
    as the stationary operand, yielding token-major out tiles.
  - Matmuls use fp32r (full PE rate, ~1e-3 relative accuracy) except the
    big aggregation matmul which uses bf16 operands (v, kern) for SBUF
    headroom.
"""

import numpy as np

import concourse.bass as bass
import concourse.tile as tile
from concourse import bacc, mybir
from concourse.bass_utils import run_bass_kernel_spmd
from concourse.masks import make_identity

F32 = mybir.dt.float32
F32R = mybir.dt.float32r
BF16 = mybir.dt.bfloat16
AF = mybir.ActivationFunctionType
OP = mybir.AluOpType

B, K, D = 8, 2048, 512
E, S = 1024, 128
F = 2 * E + S  # 2176
EPS = 1e-5
P = 128
KT = K // P    # 16 token tiles
DT = D // P    # 4  d tiles
ET = E // P    # 8  e tiles
QB = K // 512  # 4  q blocks of 512 tokens
N_CORES = 8

# dtype of the aggregation matmul operands (kern, v)
AGG_DT = BF16


def gau_tile_kernel(ctx, tc, out_d, x_d, uvwT_d, owT_d, gbT_d, rs_d, g_val, dbg=None, time_reps=1, ablate=0):
    nc = tc.nc
    inv_sqrt_s = 1.0 / float(np.sqrt(S))

    const = ctx.enter_context(tc.tile_pool(name="const", bufs=1))
    persist = ctx.enter_context(tc.tile_pool(name="persist", bufs=1))
    xwork = ctx.enter_context(tc.tile_pool(name="xwork", bufs=3))
    tmps = ctx.enter_context(tc.tile_pool(name="tmps", bufs=4))
    attn = ctx.enter_context(tc.tile_pool(name="attn", bufs=1))
    owork = ctx.enter_context(tc.tile_pool(name="owork", bufs=3))
    ps_t = ctx.enter_context(tc.tile_pool(name="ps_t", bufs=2, space="PSUM"))
    ps_mm = ctx.enter_context(tc.tile_pool(name="ps_mm", bufs=6, space="PSUM"))

    # ---- constants / weights ----
    ident = const.tile([P, P], F32)
    make_identity(nc, ident)
    gbT = const.tile([P, 4], F32)  # cols: gamma0, gamma1, beta0, beta1
    nc.sync.dma_start(gbT[:], gbT_d)
    rs_b = const.tile([P, D], F32)  # res_scale broadcast across partitions
    nc.sync.dma_start(rs_b[:], rs_d.partition_broadcast(P))

    uvw_r = uvwT_d.rearrange("(po pi) f -> pi po f", pi=P)  # [128, 4, 2176]
    uvw_u = persist.tile([P, DT, E], F32R)
    nc.sync.dma_start(uvw_u[:], uvw_r[:, :, 0:E])
    ow_r = owT_d.rearrange("(po pi) d -> pi po d", pi=P)  # [128, 8, 512]
    o_wT = persist.tile([P, ET, D], F32R)
    nc.sync.dma_start(o_wT[:], ow_r)

    xnT = persist.tile([P, DT, K], F32R)
    qT = persist.tile([P, K], F32R)
    kTt = persist.tile([P, K], F32R)
    v_sb = persist.tile([P, KT, E], AGG_DT)

    uvw_vb = persist.tile([P, DT, E + S], F32R)
    nc.sync.dma_start(uvw_vb[:], uvw_r[:, :, E:F])

    for _rep in range(time_reps):
        # ---- phases 1+2 interleaved per token tile: norm -> transpose ->
        # v-projection, then base/q/k per 4-tile group. Keeps the PE stream
        # dense (transposes + v matmuls interleave with the DVE/ACT norm
        # chain instead of serializing behind it).
        for nb in range(QB):
            for i in range(4 * nb, 4 * nb + 4):
                x_i = xwork.tile([P, D], F32, tag="x_in")
                nc.sync.dma_start(x_i[:], x_d[i * P : (i + 1) * P, :])
                xn_i = xwork.tile([P, D], F32, tag="xn")
                # sumsq via ACT square with accumulator (xn_i holds scratch x^2)
                ss = tmps.tile([P, 1], F32, tag="ss")
                nc.scalar.activation(xn_i[:], x_i[:], AF.Square, accum_out=ss[:])
                nrm = tmps.tile([P, 1], F32, tag="nrm")
                nc.scalar.activation(nrm[:], ss[:], AF.Sqrt, scale=1.0 / D)
                nc.vector.tensor_scalar_max(nrm[:], nrm[:], EPS)
                rn = tmps.tile([P, 1], F32, tag="rn")
                nc.vector.reciprocal(rn[:], nrm[:])
                nc.vector.tensor_scalar(
                    xn_i[:], x_i[:], rn[:], float(g_val), op0=OP.mult, op1=OP.mult
                )
                for j in range(DT):
                    pt = ps_t.tile([P, P], F32)
                    nc.tensor.transpose(pt[:], xn_i[:, j * P : (j + 1) * P], ident[:])
                    nc.vector.tensor_copy(xnT[:, j, i * P : (i + 1) * P], pt[:])
                # v for this token tile (weight-reuse order: j outer, halves inner)
                pv0 = ps_mm.tile([P, 512], F32, tag="mm")
                pv1 = ps_mm.tile([P, 512], F32, tag="mm")
                for j in range(DT):
                    nc.tensor.matmul(
                        pv0[:], xnT[:, j, i * P : (i + 1) * P], uvw_vb[:, j, 0:512],
                        start=(j == 0), stop=(j == DT - 1),
                    )
                    nc.tensor.matmul(
                        pv1[:], xnT[:, j, i * P : (i + 1) * P], uvw_vb[:, j, 512:1024],
                        start=(j == 0), stop=(j == DT - 1),
                    )
                nc.scalar.activation(v_sb[:, i, 0:512], pv0[:], AF.Silu)
                nc.scalar.activation(v_sb[:, i, 512:1024], pv1[:], AF.Silu)

            # base -> q, k for this 4-tile group (feature-major [S, 512])
            pb = ps_mm.tile([P, 512], F32, tag="mm")
            for j in range(DT):
                nc.tensor.matmul(
                    pb[:],
                    uvw_vb[:, j, E : E + S],
                    xnT[:, j, nb * 512 : (nb + 1) * 512],
                    start=(j == 0),
                    stop=(j == DT - 1),
                )
            sl = slice(nb * 512, (nb + 1) * 512)
            bs = owork.tile([P, 512], F32, tag="bs")
            nc.scalar.activation(bs[:], pb[:], AF.Silu)
            nc.vector.tensor_scalar(
                qT[:, sl], bs[:], gbT[:, 0:1], gbT[:, 2:3], op0=OP.mult, op1=OP.add
            )
            nc.vector.tensor_scalar(
                kTt[:, sl], bs[:], gbT[:, 1:2], gbT[:, 3:4], op0=OP.mult, op1=OP.add
            )

        if dbg is not None:
            nc.gpsimd.dma_start(dbg["xnT"], xnT[:])
            nc.gpsimd.dma_start(dbg["qT"], qT[:])
            nc.gpsimd.dma_start(dbg["kT"], kTt[:])
            nc.gpsimd.dma_start(dbg["v"], v_sb[:])

        # ---- phase 3: attention, per q-block of 512 tokens ----
        for qb in range(QB):
            qsl = slice(qb * 512, (qb + 1) * 512)
            # u for this q-block (feature-major [E, 512]), silu
            u_qb = attn.tile([P, ET, 512], AGG_DT, tag="u")
            for uf in range(ET):
                pu = ps_mm.tile([P, 512], F32, tag="mm")
                for j in range(DT):
                    nc.tensor.matmul(
                        pu[:],
                        uvw_u[:, j, uf * P : (uf + 1) * P],
                        xnT[:, j, qsl],
                        start=(j == 0),
                        stop=(j == DT - 1),
                    )
                nc.scalar.activation(u_qb[:, uf, :], pu[:], AF.Silu)

            # scoresT [k, q] -> relu(.)/sqrt(S) -> square -> kern (bf16)
            kern = attn.tile([P, KT, 512], AGG_DT, tag="kern")
            for kt in range(KT):
                psc = ps_mm.tile([P, 512], F32, tag="mm")
                nc.tensor.matmul(
                    psc[:],
                    kTt[:, kt * P : (kt + 1) * P],
                    qT[:, qsl],
                    start=True,
                    stop=True,
                )
                rt = tmps.tile([P, 512], AGG_DT, tag="relu")
                nc.scalar.activation(rt[:], psc[:], AF.Relu, scale=inv_sqrt_s)
                nc.vector.tensor_tensor(kern[:, kt, :], rt[:], rt[:], OP.mult)

            if dbg is not None and qb == 0:
                nc.gpsimd.dma_start(dbg["u0"], u_qb[:])
                nc.gpsimd.dma_start(dbg["kern0"], kern[:])

            if ablate >= 2:
                nc.gpsimd.dma_start(out_d[qb * P : (qb + 1) * P, :], kern[:, 0, :D])
                nc.gpsimd.dma_start(out_d[(4 + qb) * P : (5 + qb) * P, :], u_qb[:, 0, :D])
                nc.gpsimd.dma_start(out_d[(8 + qb) * P : (9 + qb) * P, :], v_sb[:, qb, :D])
                continue

            # aggT [e, q] += v.T-slices @ kern ; gate with u
            gated = attn.tile([P, ET, 512], F32R, tag="gated")
            for et in range(ET):
                pa = ps_mm.tile([P, 512], F32, tag="mm")
                for kt in range(KT):
                    nc.tensor.matmul(
                        pa[:],
                        v_sb[:, kt, et * P : (et + 1) * P],
                        kern[:, kt, :],
                        start=(kt == 0),
                        stop=(kt == KT - 1),
                    )
                nc.vector.tensor_tensor(gated[:, et, :], u_qb[:, et, :], pa[:], OP.mult)

            if dbg is not None and qb == 0:
                nc.gpsimd.dma_start(dbg["gated0"], gated[:])

            if ablate == 1:
                nc.gpsimd.dma_start(out_d[qb * P : (qb + 1) * P, :], gated[:, 0, :D])
                continue

            # output projection + residual, token-major
            for tq in range(4):
                i = qb * 4 + tq
                po = ps_mm.tile([P, 512], F32, tag="mm")
                for et in range(ET):
                    nc.tensor.matmul(
                        po[:],
                        gated[:, et, tq * P : (tq + 1) * P],
                        o_wT[:, et, :],
                        start=(et == 0),
                        stop=(et == ET - 1),
                    )
                x_r = owork.tile([P, D], F32, tag="x_res")
                nc.sync.dma_start(x_r[:], x_d[i * P : (i + 1) * P, :])
                ot = owork.tile([P, D], F32, tag="out")
                nc.vector.tensor_tensor(ot[:], x_r[:], rs_b[:], OP.mult)
                nc.vector.tensor_tensor(ot[:], ot[:], po[:], OP.add)
                nc.sync.dma_start(out_d[i * P : (i + 1) * P, :], ot[:])


def build_program(g_val, time_reps=1, ablate=0):
    nc = bacc.Bacc("TRN2", target_bir_lowering=False, debug=False, num_devices=N_CORES)
    x_d = nc.dram_tensor("x", [K, D], F32, kind="ExternalInput").ap()
    uvwT_d = nc.dram_tensor("uvw_t", [D, F], F32R, kind="ExternalInput").ap()
    owT_d = nc.dram_tensor("ow_t", [E, D], F32R, kind="ExternalInput").ap()
    gbT_d = nc.dram_tensor("gb_t", [P, 4], F32, kind="ExternalInput").ap()
    rs_d = nc.dram_tensor("res_scale", [D], F32, kind="ExternalInput").ap()
    out_d = nc.dram_tensor("out", [K, D], F32, kind="ExternalOutput").ap()

    from contextlib import ExitStack

    with tile.TileContext(nc) as tc, ExitStack() as ctx:
        gau_tile_kernel(
            ctx, tc, out_d, x_d, uvwT_d, owT_d, gbT_d, rs_d, g_val,
            time_reps=time_reps, ablate=ablate
        )
    nc.compile()
    return nc


_PROGRAM_CACHE = {}


def _get_program(g_val):
    key = float(g_val)
    if key not in _PROGRAM_CACHE:
        _PROGRAM_CACHE[key] = build_program(key)
    return _PROGRAM_CACHE[key]


def make_in_maps(x, uv_w, o_w, gamma, beta, res_scale):
    uvwT = np.ascontiguousarray(uv_w.T.astype(np.float32))  # [D, F]
    owT = np.ascontiguousarray(o_w.T.astype(np.float32))  # [E, D]
    gbT = np.ascontiguousarray(
        np.stack([gamma[0], gamma[1], beta[0], beta[1]], axis=1).astype(np.float32)
    )  # [S, 4]
    rs = np.ascontiguousarray(res_scale.astype(np.float32))
    return [
        {
            "x": np.ascontiguousarray(x[b].astype(np.float32)),
            "uvw_t": uvwT,
            "ow_t": owT,
            "gb_t": gbT,
            "res_scale": rs,
        }
        for b in range(N_CORES)
    ]


_EXEC_CACHE = {}


def _get_executor(nc):
    """Persistent jitted PJRT executor for `nc` (axon path) — avoids the
    per-call retrace/recompile that run_bass_via_pjrt pays. Returns a
    callable(in_maps) -> list[{name: np.ndarray}]."""
    if id(nc) in _EXEC_CACHE:
        return _EXEC_CACHE[id(nc)]

    import jax
    from jax.experimental.shard_map import shard_map
    from jax.sharding import Mesh, PartitionSpec

    from concourse.bass2jax import (
        _bass_exec_p,
        install_neuronx_cc_hook,
        partition_id_tensor,
    )

    install_neuronx_cc_hook()
    partition_name = nc.partition_id_tensor.name if nc.partition_id_tensor else None
    in_names, out_names, out_avals, zero_shapes = [], [], [], []
    for alloc in nc.m.functions[0].allocations:
        if not isinstance(alloc, mybir.MemoryLocationSet):
            continue
        name = alloc.memorylocations[0].name
        if alloc.kind == "ExternalInput":
            if name != partition_name:
                in_names.append(name)
        elif alloc.kind == "ExternalOutput":
            out_names.append(name)
            shape = tuple(alloc.tensor_shape)
            dtype = mybir.dt.np(alloc.dtype)
            out_avals.append(jax.core.ShapedArray(shape, dtype))
            zero_shapes.append((shape, dtype))
    n_params = len(in_names)
    all_names = in_names + out_names + ([partition_name] if partition_name else [])

    def _body(*args):
        operands = list(args)
        if partition_name is not None:
            operands.append(partition_id_tensor())
        return tuple(
            _bass_exec_p.bind(
                *operands,
                out_avals=tuple(out_avals),
                in_names=tuple(all_names),
                out_names=tuple(out_names),
                lowering_input_output_aliases=(),
                sim_require_finite=True,
                sim_require_nnan=True,
                nc=nc,
            )
        )

    devices = jax.devices()[:N_CORES]
    mesh = Mesh(np.asarray(devices), ("core",))
    n_zero = len(zero_shapes)
    sharded = jax.jit(
        shard_map(
            _body,
            mesh=mesh,
            in_specs=(PartitionSpec("core"),) * (n_params + n_zero),
            out_specs=(PartitionSpec("core"),) * len(out_names),
            check_rep=False,
        ),
        keep_unused=True,
    )

    def run(in_maps):
        concat_in = [
            np.concatenate(
                [np.asarray(in_maps[c][in_names[i]]) for c in range(N_CORES)], axis=0
            )
            for i in range(n_params)
        ]
        concat_zeros = [
            np.zeros((N_CORES * s[0], *s[1:]), dt) for s, dt in zero_shapes
        ]
        out_arrs = sharded(*concat_in, *concat_zeros)
        return [
            {
                name: np.asarray(out_arrs[i]).reshape(
                    N_CORES, *out_avals[i].shape
                )[c]
                for i, name in enumerate(out_names)
            }
            for c in range(N_CORES)
        ]

    _EXEC_CACHE[id(nc)] = run
    return run


def kernel(x, uv_w, o_w, gamma, beta, g, res_scale):
    x = np.asarray(x)
    nc = _get_program(float(np.asarray(g).reshape(-1)[0]))
    in_maps = make_in_maps(
        x,
        np.asarray(uv_w),
        np.asarray(o_w),
        np.asarray(gamma),
        np.asarray(beta),
        np.asarray(res_scale),
    )
    from concourse._compat import axon_active

    if axon_active():
        try:
            results = _get_executor(nc)(in_maps)
        except Exception:
            results = run_bass_kernel_spmd(
                nc, in_maps, core_ids=list(range(N_CORES))
            ).results
    else:
        results = run_bass_kernel_spmd(
            nc, in_maps, core_ids=list(range(N_CORES))
        ).results
    out = np.stack([r["out"] for r in results], axis=0)
    return out.astype(x.dtype)



# revision 19
# speedup vs baseline: 18.3977x; 1.0308x over previous
"""GAU (Gated Attention Unit) encoder kernel for Trainium2, 8 NeuronCores.

Reference computation (per sample, B=8 samples total, one per core):
    xn   = ScaleNorm(x) * g                          # [K, D]
    uv   = silu(xn @ uv_w.T)                         # [K, 2E+S]
    u, v, base = split(uv, [E, E, S])
    q, k = base * gamma[i] + beta[i]                 # [K, S] each
    kern = relu(q @ k.T / sqrt(S))^2                 # [K, K]
    out  = (u * (kern @ v)) @ o_w.T + x * res_scale  # [K, D]

Sharding: data-parallel over batch B — one sample per NeuronCore (SPMD,
no collectives). Weights replicated.

Layout strategy (per core):
  - x arrives token-major [K, D]; ScaleNorm is a free-dim reduction.
  - xn is transposed on-chip via PE (64 128x128 transposes) to xnT [D, K],
    which feeds every uv-projection matmul (contraction over D needs D on
    partitions).
  - q, k, base, u are produced feature-major ([S|E, K]); v token-major
    [K, E]. Scores are computed transposed (scoresT [k, q]) so the
    aggregation matmul can contract over k with no further transposes, and
    the gated output (feature-major) directly feeds the output projection
    as the stationary operand, yielding token-major out tiles.
  - Matmuls use fp32r (full PE rate, ~1e-3 relative accuracy) except the
    big aggregation matmul which uses bf16 operands (v, kern) for SBUF
    headroom.
"""

import numpy as np

import concourse.bass as bass
import concourse.tile as tile
from concourse import bacc, mybir
from concourse.bass_utils import run_bass_kernel_spmd
from concourse.masks import make_identity

F32 = mybir.dt.float32
F32R = mybir.dt.float32r
BF16 = mybir.dt.bfloat16
AF = mybir.ActivationFunctionType
OP = mybir.AluOpType

B, K, D = 8, 2048, 512
E, S = 1024, 128
F = 2 * E + S  # 2176
EPS = 1e-5
P = 128
KT = K // P    # 16 token tiles
DT = D // P    # 4  d tiles
ET = E // P    # 8  e tiles
QB = K // 512  # 4  q blocks of 512 tokens
N_CORES = 8

# dtype of the aggregation matmul operands (kern, v)
AGG_DT = BF16


def gau_tile_kernel(ctx, tc, out_d, x_d, uvwT_d, owT_d, gbT_d, rs_d, g_val, dbg=None, time_reps=1, ablate=0):
    nc = tc.nc
    inv_sqrt_s = 1.0 / float(np.sqrt(S))

    const = ctx.enter_context(tc.tile_pool(name="const", bufs=1))
    persist = ctx.enter_context(tc.tile_pool(name="persist", bufs=1))
    xwork = ctx.enter_context(tc.tile_pool(name="xwork", bufs=3))
    tmps = ctx.enter_context(tc.tile_pool(name="tmps", bufs=4))
    attn = ctx.enter_context(tc.tile_pool(name="attn", bufs=1))
    owork = ctx.enter_context(tc.tile_pool(name="owork", bufs=3))
    ps_t = ctx.enter_context(tc.tile_pool(name="ps_t", bufs=2, space="PSUM"))
    ps_mm = ctx.enter_context(tc.tile_pool(name="ps_mm", bufs=6, space="PSUM"))

    # ---- constants / weights ----
    ident = const.tile([P, P], F32)
    make_identity(nc, ident)
    gbT = const.tile([P, 4], F32)  # cols: gamma0, gamma1, beta0, beta1
    nc.sync.dma_start(gbT[:], gbT_d)
    rs_b = const.tile([P, D], F32)  # res_scale broadcast across partitions
    nc.sync.dma_start(rs_b[:], rs_d.partition_broadcast(P))

    uvw_r = uvwT_d.rearrange("(po pi) f -> pi po f", pi=P)  # [128, 4, 2176]
    uvw_u = persist.tile([P, DT, E], F32R)
    nc.sync.dma_start(uvw_u[:], uvw_r[:, :, 0:E])
    ow_r = owT_d.rearrange("(po pi) d -> pi po d", pi=P)  # [128, 8, 512]
    o_wT = persist.tile([P, ET, D], F32R)
    nc.sync.dma_start(o_wT[:], ow_r)

    xnT = persist.tile([P, DT, K], F32R)
    qT = persist.tile([P, K], F32R)
    kTt = persist.tile([P, K], F32R)
    v_sb = persist.tile([P, KT, E], AGG_DT)

    uvw_vb = persist.tile([P, DT, E + S], F32R)
    nc.sync.dma_start(uvw_vb[:], uvw_r[:, :, E:F])

    for _rep in range(time_reps):
        # ---- phases 1+2 interleaved per token tile: norm -> transpose ->
        # v-projection, then base/q/k per 4-tile group. Keeps the PE stream
        # dense (transposes + v matmuls interleave with the DVE/ACT norm
        # chain instead of serializing behind it).
        for nb in range(QB):
            for i in range(4 * nb, 4 * nb + 4):
                x_i = xwork.tile([P, D], F32, tag="x_in")
                nc.sync.dma_start(x_i[:], x_d[i * P : (i + 1) * P, :])
                xn_i = xwork.tile([P, D], F32, tag="xn")
                # sumsq via ACT square with accumulator (xn_i holds scratch x^2)
                ss = tmps.tile([P, 1], F32, tag="ss")
                nc.scalar.activation(xn_i[:], x_i[:], AF.Square, accum_out=ss[:])
                nrm = tmps.tile([P, 1], F32, tag="nrm")
                nc.scalar.activation(nrm[:], ss[:], AF.Sqrt, scale=1.0 / D)
                nc.vector.tensor_scalar_max(nrm[:], nrm[:], EPS)
                rn = tmps.tile([P, 1], F32, tag="rn")
                nc.vector.reciprocal(rn[:], nrm[:])
                nc.vector.tensor_scalar(
                    xn_i[:], x_i[:], rn[:], float(g_val), op0=OP.mult, op1=OP.mult
                )
                pt = ps_t.tile([P, 512], F32)
                for j in range(DT):
                    nc.tensor.transpose(
                        pt[:, j * P : (j + 1) * P],
                        xn_i[:, j * P : (j + 1) * P],
                        ident[:],
                    )
                nc.vector.tensor_copy(
                    xnT[:, :, i * P : (i + 1) * P],
                    pt.rearrange("p (j c) -> p j c", c=P),
                )
                # v for this token tile (weight-reuse order: j outer, halves inner)
                pv0 = ps_mm.tile([P, 512], F32, tag="mm")
                pv1 = ps_mm.tile([P, 512], F32, tag="mm")
                for j in range(DT):
                    nc.tensor.matmul(
                        pv0[:], xnT[:, j, i * P : (i + 1) * P], uvw_vb[:, j, 0:512],
                        start=(j == 0), stop=(j == DT - 1),
                    )
                    nc.tensor.matmul(
                        pv1[:], xnT[:, j, i * P : (i + 1) * P], uvw_vb[:, j, 512:1024],
                        start=(j == 0), stop=(j == DT - 1),
                    )
                nc.scalar.activation(v_sb[:, i, 0:512], pv0[:], AF.Silu)
                nc.scalar.activation(v_sb[:, i, 512:1024], pv1[:], AF.Silu)

            # base -> q, k for this 4-tile group (feature-major [S, 512])
            pb = ps_mm.tile([P, 512], F32, tag="mm")
            for j in range(DT):
                nc.tensor.matmul(
                    pb[:],
                    uvw_vb[:, j, E : E + S],
                    xnT[:, j, nb * 512 : (nb + 1) * 512],
                    start=(j == 0),
                    stop=(j == DT - 1),
                )
            sl = slice(nb * 512, (nb + 1) * 512)
            bs = owork.tile([P, 512], F32, tag="bs")
            nc.scalar.activation(bs[:], pb[:], AF.Silu)
            nc.vector.tensor_scalar(
                qT[:, sl], bs[:], gbT[:, 0:1], gbT[:, 2:3], op0=OP.mult, op1=OP.add
            )
            nc.vector.tensor_scalar(
                kTt[:, sl], bs[:], gbT[:, 1:2], gbT[:, 3:4], op0=OP.mult, op1=OP.add
            )

        if ablate >= 3:
            # consume p1+p2 products so nothing is dead-code eliminated
            nc.gpsimd.dma_start(out_d[0:P, :], qT[:, 0:D])
            nc.gpsimd.dma_start(out_d[P : 2 * P, :], kTt[:, 0:D])
            nc.gpsimd.dma_start(out_d[2 * P : 3 * P, :], v_sb[:, 0, 0:D])
            nc.gpsimd.dma_start(out_d[3 * P : 4 * P, :], xnT[:, 0, 0:D])
            continue

        if dbg is not None:
            nc.gpsimd.dma_start(dbg["xnT"], xnT[:])
            nc.gpsimd.dma_start(dbg["qT"], qT[:])
            nc.gpsimd.dma_start(dbg["kT"], kTt[:])
            nc.gpsimd.dma_start(dbg["v"], v_sb[:])

        # ---- phase 3: attention, per q-block of 512 tokens ----
        for qb in range(QB):
            qsl = slice(qb * 512, (qb + 1) * 512)
            # u for this q-block (feature-major [E, 512]), silu
            u_qb = attn.tile([P, ET, 512], AGG_DT, tag="u")
            for uf in range(ET):
                pu = ps_mm.tile([P, 512], F32, tag="mm")
                for j in range(DT):
                    nc.tensor.matmul(
                        pu[:],
                        uvw_u[:, j, uf * P : (uf + 1) * P],
                        xnT[:, j, qsl],
                        start=(j == 0),
                        stop=(j == DT - 1),
                    )
                nc.scalar.activation(u_qb[:, uf, :], pu[:], AF.Silu)

            # scoresT [k, q] -> relu(.)/sqrt(S) -> square -> kern (bf16)
            kern = attn.tile([P, KT, 512], AGG_DT, tag="kern")
            for kt in range(KT):
                psc = ps_mm.tile([P, 512], F32, tag="mm")
                nc.tensor.matmul(
                    psc[:],
                    kTt[:, kt * P : (kt + 1) * P],
                    qT[:, qsl],
                    start=True,
                    stop=True,
                )
                rt = tmps.tile([P, 512], AGG_DT, tag="relu")
                nc.scalar.activation(rt[:], psc[:], AF.Relu, scale=inv_sqrt_s)
                nc.vector.tensor_tensor(kern[:, kt, :], rt[:], rt[:], OP.mult)

            if dbg is not None and qb == 0:
                nc.gpsimd.dma_start(dbg["u0"], u_qb[:])
                nc.gpsimd.dma_start(dbg["kern0"], kern[:])

            if ablate >= 2:
                nc.gpsimd.dma_start(out_d[qb * P : (qb + 1) * P, :], kern[:, 0, :D])
                nc.gpsimd.dma_start(out_d[(4 + qb) * P : (5 + qb) * P, :], u_qb[:, 0, :D])
                nc.gpsimd.dma_start(out_d[(8 + qb) * P : (9 + qb) * P, :], v_sb[:, qb, :D])
                continue

            # aggT [e, q] += v.T-slices @ kern ; gate with u
            gated = attn.tile([P, ET, 512], F32R, tag="gated")
            for et in range(ET):
                pa = ps_mm.tile([P, 512], F32, tag="mm")
                for kt in range(KT):
                    nc.tensor.matmul(
                        pa[:],
                        v_sb[:, kt, et * P : (et + 1) * P],
                        kern[:, kt, :],
                        start=(kt == 0),
                        stop=(kt == KT - 1),
                    )
                nc.vector.tensor_tensor(gated[:, et, :], u_qb[:, et, :], pa[:], OP.mult)

            if dbg is not None and qb == 0:
                nc.gpsimd.dma_start(dbg["gated0"], gated[:])

            if ablate == 1:
                nc.gpsimd.dma_start(out_d[qb * P : (qb + 1) * P, :], gated[:, 0, :D])
                continue

            # output projection + residual, token-major
            for tq in range(4):
                i = qb * 4 + tq
                po = ps_mm.tile([P, 512], F32, tag="mm")
                for et in range(ET):
                    nc.tensor.matmul(
                        po[:],
                        gated[:, et, tq * P : (tq + 1) * P],
                        o_wT[:, et, :],
                        start=(et == 0),
                        stop=(et == ET - 1),
                    )
                x_r = owork.tile([P, D], F32, tag="x_res")
                nc.sync.dma_start(x_r[:], x_d[i * P : (i + 1) * P, :])
                ot = owork.tile([P, D], F32, tag="out")
                nc.vector.tensor_tensor(ot[:], x_r[:], rs_b[:], OP.mult)
                nc.vector.tensor_tensor(ot[:], ot[:], po[:], OP.add)
                nc.sync.dma_start(out_d[i * P : (i + 1) * P, :], ot[:])


def build_program(g_val, time_reps=1, ablate=0):
    nc = bacc.Bacc("TRN2", target_bir_lowering=False, debug=False, num_devices=N_CORES)
    x_d = nc.dram_tensor("x", [K, D], F32, kind="ExternalInput").ap()
    uvwT_d = nc.dram_tensor("uvw_t", [D, F], F32R, kind="ExternalInput").ap()
    owT_d = nc.dram_tensor("ow_t", [E, D], F32R, kind="ExternalInput").ap()
    gbT_d = nc.dram_tensor("gb_t", [P, 4], F32, kind="ExternalInput").ap()
    rs_d = nc.dram_tensor("res_scale", [D], F32, kind="ExternalInput").ap()
    out_d = nc.dram_tensor("out", [K, D], F32, kind="ExternalOutput").ap()

    from contextlib import ExitStack

    with tile.TileContext(nc) as tc, ExitStack() as ctx:
        gau_tile_kernel(
            ctx, tc, out_d, x_d, uvwT_d, owT_d, gbT_d, rs_d, g_val,
            time_reps=time_reps, ablate=ablate
        )
    nc.compile()
    return nc


_PROGRAM_CACHE = {}


def _get_program(g_val):
    key = float(g_val)
    if key not in _PROGRAM_CACHE:
        _PROGRAM_CACHE[key] = build_program(key)
    return _PROGRAM_CACHE[key]


def make_in_maps(x, uv_w, o_w, gamma, beta, res_scale):
    uvwT = np.ascontiguousarray(uv_w.T.astype(np.float32))  # [D, F]
    owT = np.ascontiguousarray(o_w.T.astype(np.float32))  # [E, D]
    gbT = np.ascontiguousarray(
        np.stack([gamma[0], gamma[1], beta[0], beta[1]], axis=1).astype(np.float32)
    )  # [S, 4]
    rs = np.ascontiguousarray(res_scale.astype(np.float32))
    return [
        {
            "x": np.ascontiguousarray(x[b].astype(np.float32)),
            "uvw_t": uvwT,
            "ow_t": owT,
            "gb_t": gbT,
            "res_scale": rs,
        }
        for b in range(N_CORES)
    ]


_EXEC_CACHE = {}


def _get_executor(nc):
    """Persistent jitted PJRT executor for `nc` (axon path) — avoids the
    per-call retrace/recompile that run_bass_via_pjrt pays. Returns a
    callable(in_maps) -> list[{name: np.ndarray}]."""
    if id(nc) in _EXEC_CACHE:
        return _EXEC_CACHE[id(nc)]

    import jax
    from jax.experimental.shard_map import shard_map
    from jax.sharding import Mesh, PartitionSpec

    from concourse.bass2jax import (
        _bass_exec_p,
        install_neuronx_cc_hook,
        partition_id_tensor,
    )

    install_neuronx_cc_hook()
    partition_name = nc.partition_id_tensor.name if nc.partition_id_tensor else None
    in_names, out_names, out_avals, zero_shapes = [], [], [], []
    for alloc in nc.m.functions[0].allocations:
        if not isinstance(alloc, mybir.MemoryLocationSet):
            continue
        name = alloc.memorylocations[0].name
        if alloc.kind == "ExternalInput":
            if name != partition_name:
                in_names.append(name)
        elif alloc.kind == "ExternalOutput":
            out_names.append(name)
            shape = tuple(alloc.tensor_shape)
            dtype = mybir.dt.np(alloc.dtype)
            out_avals.append(jax.core.ShapedArray(shape, dtype))
            zero_shapes.append((shape, dtype))
    n_params = len(in_names)
    all_names = in_names + out_names + ([partition_name] if partition_name else [])

    def _body(*args):
        operands = list(args)
        if partition_name is not None:
            operands.append(partition_id_tensor())
        return tuple(
            _bass_exec_p.bind(
                *operands,
                out_avals=tuple(out_avals),
                in_names=tuple(all_names),
                out_names=tuple(out_names),
                lowering_input_output_aliases=(),
                sim_require_finite=True,
                sim_require_nnan=True,
                nc=nc,
            )
        )

    devices = jax.devices()[:N_CORES]
    mesh = Mesh(np.asarray(devices), ("core",))
    n_zero = len(zero_shapes)
    sharded = jax.jit(
        shard_map(
            _body,
            mesh=mesh,
            in_specs=(PartitionSpec("core"),) * (n_params + n_zero),
            out_specs=(PartitionSpec("core"),) * len(out_names),
            check_rep=False,
        ),
        keep_unused=True,
    )

    def run(in_maps):
        concat_in = [
            np.concatenate(
                [np.asarray(in_maps[c][in_names[i]]) for c in range(N_CORES)], axis=0
            )
            for i in range(n_params)
        ]
        concat_zeros = [
            np.zeros((N_CORES * s[0], *s[1:]), dt) for s, dt in zero_shapes
        ]
        out_arrs = sharded(*concat_in, *concat_zeros)
        return [
            {
                name: np.asarray(out_arrs[i]).reshape(
                    N_CORES, *out_avals[i].shape
                )[c]
                for i, name in enumerate(out_names)
            }
            for c in range(N_CORES)
        ]

    _EXEC_CACHE[id(nc)] = run
    return run


def kernel(x, uv_w, o_w, gamma, beta, g, res_scale):
    x = np.asarray(x)
    nc = _get_program(float(np.asarray(g).reshape(-1)[0]))
    in_maps = make_in_maps(
        x,
        np.asarray(uv_w),
        np.asarray(o_w),
        np.asarray(gamma),
        np.asarray(beta),
        np.asarray(res_scale),
    )
    from concourse._compat import axon_active

    if axon_active():
        try:
            results = _get_executor(nc)(in_maps)
        except Exception:
            results = run_bass_kernel_spmd(
                nc, in_maps, core_ids=list(range(N_CORES))
            ).results
    else:
        results = run_bass_kernel_spmd(
            nc, in_maps, core_ids=list(range(N_CORES))
        ).results
    out = np.stack([r["out"] for r in results], axis=0)
    return out.astype(x.dtype)

